# revision 27
# baseline (speedup 1.0000x reference)
"""BatchAllTripletLoss Trainium2 kernel.

Problem (hardcoded): x (64, 256, 256) f32, y (64, 256) int64 with
y[p, i] = i // 8 (32 classes x 8 members, uniform, identical across parts).
Output: per-part batch-all triplet loss, shape (64,) f32.

Math per part:
  D[i,j]  = euclidean distance matrix from x_p
  pos(i)  = 8 same-class columns (incl. self), neg(i) = 248 others
  loss_p  = mean over nonzero of relu(margin + D[i,j] - D[i,l]),
            j in pos(i), l in neg(i)

Device strategy (8 NeuronCores, 8 parts each, fully independent):
  - all x row loads prefetched up front on the SP DMA queue (they are
    dependency-free, so no head-of-line blocking); per part: cast to
    bf16 on Pool, transpose via DMA xbar (dma_start_transpose) -- the
    PE does no transposes.
  - gram via PE (bf16); -sq_row/2 rank-1 and -L/2 on same-class columns
    (rank-32) folded into one PSUM group; ACT sqrt (scale=-2,
    bias=sq_col) -> D' in BF16 (pos columns polluted to ~1024).
  - own-class raw grams sit at -L/2 in psum1, so a DVE min-reduce
    extracts them with NO second PSUM accumulation; true pos dists via
    one small clamped sqrt (clamp also guards the self slot, replacing
    the diagonal-eps matmul).
  - epilogue per (half, pos-slot): one sum pass + one count pass.
    DVE sum: accum Sum_l min(D', pm) (op1=add is the accum reducer);
    relu-sum recovered on the host as 256*pm_sum - Sum(min), with the
    per-anchor pm sums riding the same PE column-sum via pm columns in
    the acc tile.  Counts: DVE is_lt (2 slots) + ACT Sign (6 slots),
    balancing measured engine rates (DVE ~390ns/pass at the 1x
    CACHE_REDUCE rate -- accumulating tensor_scalar never enters the
    2x/4x DVE modes -- vs ACT ~590ns incl. accumulator read).
  - per-core output: raw [1, 384] column sums; host finishes the
    (S_p, N_p) reconstruction and division.
"""

import numpy as np
from contextlib import ExitStack

import concourse.bass as bass
import concourse.bacc as bacc_mod
import concourse.mybir as mybir
import concourse.tile as tile

F32 = mybir.dt.float32
BF16 = mybir.dt.bfloat16
ALU = mybir.AluOpType
ACTF = mybir.ActivationFunctionType

# problem constants
P_TOT, N, C = 64, 256, 256
K, NCLS = 8, 32
MARGIN = 0.2
NCORES = 8
PPC = P_TOT // NCORES  # parts per core
HALVES = 2
LBIG = float(2 << 19)  # 2^20 pollution offset
EPS2 = 0.04  # clamp floor for pos squared-dists (self slot)

# epilogue slot -> engine maps ('V' dve, 'A' act). Pool's tensor_scalar
# with accum_out is rejected by the TRN2 ISA, so Pool handles casts only.
# Group order must be V..., A... (finalize reduces contiguous ranges).
SUM_ENG = ["V", "V", "V", "V", "V", "V", "V", "V"]
CNT_ENG = ["V", "V", "A", "A", "A", "A", "A", "A"]
N_V_SUM = sum(1 for e in SUM_ENG if e == "V")
N_V_CNT = sum(1 for e in CNT_ENG if e == "V")
N_A_SUM = K - N_V_SUM
N_A_CNT = K - N_V_CNT
HW = 24  # per-half acc cols: 0-7 sums, 8-15 counts, 16-23 pm
ACC_W = 2 * HW


def build_kernel(do_compile=True, reps=1):
    nc = bacc_mod.Bacc()
    x_in = nc.declare_dram_parameter("x", [PPC * N, C], F32, isOutput=False)
    sn_out = nc.declare_dram_parameter("sn", [1, ACC_W * PPC], F32, isOutput=True)

    with tile.TileContext(nc) as tc, ExitStack() as ctx:
        consts = ctx.enter_context(tc.tile_pool(name="consts", bufs=1))
        xpool = ctx.enter_context(tc.tile_pool(name="xpool", bufs=2))
        xtpool = ctx.enter_context(tc.tile_pool(name="xtpool", bufs=4))
        dpool = ctx.enter_context(tc.tile_pool(name="dpool", bufs=5))
        small = ctx.enter_context(tc.tile_pool(name="small", bufs=8))
        trash = ctx.enter_context(tc.tile_pool(name="trash", bufs=12))
        accp = ctx.enter_context(tc.tile_pool(name="accp", bufs=5))
        psum = ctx.enter_context(tc.tile_pool(name="psum", bufs=6, space="PSUM"))
        psmall = ctx.enter_context(tc.tile_pool(name="psmall", bufs=1, space="PSUM"))

        # ---- one-time constants ----
        ct_one = consts.tile([NCLS, N], BF16, tag="ct1")
        nc.vector.memset(ct_one[:], 1.0)
        nc.gpsimd.affine_select(
            ct_one[:], ct_one[:], pattern=[[1, NCLS], [0, K]],
            compare_op=ALU.is_equal, fill=0.0, base=0, channel_multiplier=-1,
        )
        ct_a = consts.tile([NCLS, N], BF16, tag="cta")  # -L/2 * B
        nc.vector.memset(ct_a[:], -LBIG / 2)
        nc.gpsimd.affine_select(
            ct_a[:], ct_a[:], pattern=[[1, NCLS], [0, K]],
            compare_op=ALU.is_equal, fill=0.0, base=0, channel_multiplier=-1,
        )
        ident = consts.tile([128, 128], BF16, tag="ident")
        nc.vector.memset(ident[:], 1.0)
        nc.gpsimd.affine_select(
            ident[:], ident[:], pattern=[[1, 128]],
            compare_op=ALU.is_equal, fill=0.0, base=0, channel_multiplier=-1,
        )
        neghalf = consts.tile([1, 128], BF16, tag="neghalf")
        nc.vector.memset(neghalf[:], -0.5)
        ones_col = consts.tile([128, 1], F32, tag="ones_col")
        nc.vector.memset(ones_col[:], 1.0)

        # persistent cross-part psum strip
        fin_ps = psmall.tile([1, ACC_W * PPC], F32, tag="fin_ps")

        # ---- prefetch all input loads up front (dependency-free, so no
        # head-of-line blocking on the in-order SP queue); casts and xbar
        # transposes stay inline with each part's compute
        NPARTS = reps * PPC
        PRELOAD = min(8, NPARTS)
        parts = [pp for _ in range(reps) for pp in range(PPC)]
        xfs = {}

        def load(pi):
            p = parts[pi]
            xf = [xpool.tile([128, C], F32, tag="xf", name="xf", bufs=2 * (PRELOAD + 2))
                  for _ in range(HALVES)]
            for h in range(HALVES):
                nc.sync.dma_start(xf[h][:], x_in[p * N + 128 * h: p * N + 128 * (h + 1), :])
            xfs[pi] = xf

        for pi in range(PRELOAD):
            load(pi)

        pending_colsum = None
        for pi, p in enumerate(parts):
            xf = xfs.pop(pi)
            xb = [xpool.tile([128, C], BF16, tag="xb", name="xb", bufs=6)
                  for _ in range(HALVES)]
            for h in range(HALVES):
                nc.gpsimd.tensor_copy(xb[h][:], xf[h][:])
            xtb_all = xtpool.tile([128, 2 * N], BF16, tag="xtb", name="xtb", bufs=4)
            xtb_v = xtb_all[:].rearrange("q (a b) -> q a b", a=2, b=N)
            for h in range(HALVES):
                nc.sync.dma_start_transpose(
                    xtb_v[:, :, 128 * h: 128 * (h + 1)], xb[h][:]
                )
            xtb = [xtb_all[:, 0:N], xtb_all[:, N: 2 * N]]
            if pi + PRELOAD < NPARTS:
                load(pi + PRELOAD)

            # ---- squared norms: sqcol (f32, DVE mul-reduce) + sqrow bf16 ----
            sqcol = []
            for h in range(HALVES):
                sc = small.tile([128, 1], F32, tag="sqcol")
                st = trash.tile([128, C], BF16, tag="tr_sq")
                nc.scalar.activation(st[:], xb[h][:], ACTF.Square, accum_out=sc[:])
                sqcol.append(sc)
            sqcol_b = []
            for h in range(HALVES):
                scb = small.tile([128, 1], BF16, tag="sqcolb")
                nc.vector.tensor_copy(scb[:], sqcol[h][:])
                sqcol_b.append(scb)
            sqrow_ps = psmall.tile([1, N], BF16, tag="sqrow_ps")
            for h in range(HALVES):
                nc.tensor.transpose(
                    sqrow_ps[0:1, 128 * h: 128 * (h + 1)], sqcol_b[h][:], ident[:]
                )
            sqrow = small.tile([1, N], BF16, tag="sqrow")
            nc.vector.tensor_copy(sqrow[:], sqrow_ps[:])

            # ---- per half: gram + pollution; sqrt -> D' bf16; flip; spos ----
            acc = accp.tile([128, ACC_W], F32, tag="acc", name="acc")
            dmat = []
            argc = small.tile([128, 2 * K], F32, tag="argc")
            for h in range(HALVES):
                ps = psum.tile([128, N], F32, tag="ps")
                nc.tensor.matmul(
                    ps[:], xtb[0][:, 128 * h: 128 * (h + 1)], xtb[0][:],
                    start=True, stop=False,
                )
                nc.tensor.matmul(
                    ps[:], xtb[1][:, 128 * h: 128 * (h + 1)], xtb[1][:],
                    start=False, stop=False,
                )
                nc.tensor.matmul(
                    ps[:], neghalf[:, 0:128], sqrow[:], start=False, stop=False,
                )
                nc.tensor.matmul(
                    ps[:], ct_a[:, 128 * h: 128 * (h + 1)], ct_one[:],
                    start=False, stop=True,
                )
                dm = dpool.tile([128, N], BF16, tag="dmat")
                nc.scalar.activation(
                    dm[:], ps[:], ACTF.Sqrt, bias=sqcol[h][:], scale=-2.0,
                )
                dmat.append(dm)

                # own-class raw grams sit at -L/2 in psum1: min-reduce
                # picks them without any flip matmul (sqrt runs in parallel)
                spos = small.tile([128, K], F32, tag="spos")
                nc.vector.tensor_reduce(
                    spos[:],
                    ps[:].rearrange("q (g t) -> q t g", g=NCLS, t=K),
                    axis=mybir.AxisListType.X, op=ALU.min,
                )
                # argc[:, 8h:8h+8] = sqcol - 2*spos  (squared pos dists)
                nc.vector.tensor_scalar(
                    argc[:, K * h: K * (h + 1)], spos[:], -2.0, sqcol[h][:],
                    op0=ALU.mult, op1=ALU.add,
                )
            # previous part's acc column-sum goes on the PE queue here --
            # after this part's gram matmuls -- so those never stall behind
            # a colsum that is still waiting on the old epilogue's accums
            if pending_colsum is not None:
                prev_p, prev_acc = pending_colsum
                nc.tensor.matmul(
                    fin_ps[0:1, ACC_W * prev_p: ACC_W * (prev_p + 1)],
                    ones_col[:], prev_acc[:], start=True, stop=True,
                )

            # undo the -L/2 pollution (scale -2 => +L), clamp, sqrt, margin
            nc.vector.tensor_scalar(argc[:], argc[:], LBIG, EPS2, op0=ALU.subtract, op1=ALU.max)
            pm = small.tile([128, 2 * K], F32, tag="pm")
            nc.scalar.activation(pm[:], argc[:], ACTF.Sqrt)
            nc.vector.tensor_scalar(pm[:], pm[:], MARGIN, None, op0=ALU.add)
            # stash pm into acc cols 16-23 per half so the PE column-sum
            # yields per-(p,h,t) threshold sums for the finalize correction
            nc.vector.tensor_copy(
                acc[:].rearrange("q (h c) -> q h c", h=2, c=HW)[:, :, 16: 16 + K],
                pm[:].rearrange("q (h t) -> q h t", h=2, t=K),
            )

            # ---- epilogue: per (half, slot) one sum pass + one count pass ----
            for h in range(HALVES):
                pmh = pm[:, K * h: K * (h + 1)]
                for t in range(K):
                    a_sum = acc[:, HW * h + t: HW * h + t + 1]
                    a_cnt = acc[:, HW * h + 8 + t: HW * h + 9 + t]
                    if SUM_ENG[t] == "V":
                        o = trash.tile([128, N], BF16, tag="trS")
                        nc.vector.tensor_scalar(
                            o[:], dmat[h][:], pmh[:, t: t + 1], None,
                            op0=ALU.min, op1=ALU.add, accum_out=a_sum,
                        )
                    else:
                        o = trash.tile([128, N], BF16, tag="trS")
                        nc.scalar.activation(
                            o[:], dmat[h][:], ACTF.Relu,
                            bias=pmh[:, t: t + 1], scale=-1.0, accum_out=a_sum,
                        )
                    if CNT_ENG[t] == "V":
                        o = trash.tile([128, N], BF16, tag="trC")
                        nc.vector.tensor_scalar(
                            o[:], dmat[h][:], pmh[:, t: t + 1], None,
                            op0=ALU.is_lt, op1=ALU.add, accum_out=a_cnt,
                        )
                    else:
                        o = trash.tile([128, N], BF16, tag="trC")
                        nc.scalar.activation(
                            o[:], dmat[h][:], ACTF.Sign,
                            bias=pmh[:, t: t + 1], scale=-1.0, accum_out=a_cnt,
                        )

            # ---- cross-partition col sums via PE (deferred one part) ----
            pending_colsum = (p, acc)

        if pending_colsum is not None:
            prev_p, prev_acc = pending_colsum
            nc.tensor.matmul(
                fin_ps[0:1, ACC_W * prev_p: ACC_W * (prev_p + 1)],
                ones_col[:], prev_acc[:], start=True, stop=True,
            )

        # ---- dump raw per-part column sums; host does the final math ----
        fin = small.tile([1, ACC_W * PPC], F32, tag="fin")
        nc.vector.tensor_copy(fin[:], fin_ps[:])
        nc.sync.dma_start(sn_out[:], fin[:])

    if do_compile:
        nc.compile()
    return nc


_NC_CACHE = None


def _get_nc():
    global _NC_CACHE
    if _NC_CACHE is None:
        _NC_CACHE = build_kernel()
    return _NC_CACHE


def kernel(x: np.ndarray, y: np.ndarray) -> np.ndarray:
    from concourse.bass_utils import run_bass_kernel_spmd

    x = np.asarray(x)
    y = np.asarray(y)
    assert x.shape == (P_TOT, N, C) and y.shape == (P_TOT, N)
    expect = np.repeat(np.arange(NCLS, dtype=np.int64), K)
    assert np.array_equal(y, np.broadcast_to(expect, (P_TOT, N))), (
        "kernel requires y[p, i] == i // 8"
    )
    nc = _get_nc()
    xs = np.ascontiguousarray(x.reshape(NCORES, PPC * N, C).astype(np.float32))
    in_maps = [{"x": xs[i]} for i in range(NCORES)]
    res = run_bass_kernel_spmd(nc, in_maps, list(range(NCORES)))
    out = np.empty((P_TOT,), np.float32)
    for i in range(NCORES):
        fin = res.results[i]["sn"].reshape(PPC, HALVES, HW).astype(np.float64)
        sums = fin[:, :, 0:8]
        cnts = fin[:, :, 8:16]
        pms = fin[:, :, 16:24]
        S = (N * pms[:, :, :N_V_SUM].sum(2) - sums[:, :, :N_V_SUM].sum(2))
        if N_A_SUM:
            S += sums[:, :, N_V_SUM:].sum(2)
        Nc = cnts[:, :, :N_V_CNT].sum(2)
        if N_A_CNT:
            Nc += (cnts[:, :, N_V_CNT:].sum(2) + 128 * N * N_A_CNT) / 2
        S_p = S.sum(1)
        N_p = Nc.sum(1)
        out[i * PPC: (i + 1) * PPC] = np.where(
            N_p <= 0, 0.0, S_p / np.maximum(N_p, 1.0)
        ).astype(np.float32)
    return out


# revision 28
# speedup vs baseline: 1.0433x; 1.0433x over previous
"""BatchAllTripletLoss Trainium2 kernel.

Problem (hardcoded): x (64, 256, 256) f32, y (64, 256) int64 with
y[p, i] = i // 8 (32 classes x 8 members, uniform, identical across parts).
Output: per-part batch-all triplet loss, shape (64,) f32.

Math per part:
  D[i,j]  = euclidean distance matrix from x_p
  pos(i)  = 8 same-class columns (incl. self), neg(i) = 248 others
  loss_p  = mean over nonzero of relu(margin + D[i,j] - D[i,l]),
            j in pos(i), l in neg(i)

Device strategy (8 NeuronCores, 8 parts each, fully independent):
  - all x row loads prefetched up front on the SP DMA queue (they are
    dependency-free, so no head-of-line blocking); per part: cast to
    bf16 on Pool, transpose via DMA xbar (dma_start_transpose) -- the
    PE does no transposes.
  - gram via PE (bf16); -sq_row/2 rank-1 and -L/2 on same-class columns
    (rank-32) folded into one PSUM group; ACT sqrt (scale=-2,
    bias=sq_col) -> D' in BF16 (pos columns polluted to ~1024).
  - own-class raw grams sit at -L/2 in psum1, so a DVE min-reduce
    extracts them with NO second PSUM accumulation; true pos dists via
    one small clamped sqrt (clamp also guards the self slot, replacing
    the diagonal-eps matmul).
  - epilogue per (half, pos-slot): one sum pass + one count pass.
    DVE sum: accum Sum_l min(D', pm) (op1=add is the accum reducer);
    relu-sum recovered on the host as 256*pm_sum - Sum(min), with the
    per-anchor pm sums riding the same PE column-sum via pm columns in
    the acc tile.  Counts: DVE is_lt (2 slots) + ACT Sign (6 slots),
    balancing measured engine rates (DVE ~390ns/pass at the 1x
    CACHE_REDUCE rate -- accumulating tensor_scalar never enters the
    2x/4x DVE modes -- vs ACT ~590ns incl. accumulator read).
  - per-core output: raw [1, 384] column sums; host finishes the
    (S_p, N_p) reconstruction and division.
"""

import numpy as np
from contextlib import ExitStack

import concourse.bass as bass
import concourse.bacc as bacc_mod
import concourse.mybir as mybir
import concourse.tile as tile

F32 = mybir.dt.float32
BF16 = mybir.dt.bfloat16
ALU = mybir.AluOpType
ACTF = mybir.ActivationFunctionType

# problem constants
P_TOT, N, C = 64, 256, 256
K, NCLS = 8, 32
MARGIN = 0.2
NCORES = 8
PPC = P_TOT // NCORES  # parts per core
HALVES = 2
LBIG = float(2 << 19)  # 2^20 pollution offset
EPS2 = 0.04  # clamp floor for pos squared-dists (self slot)

# epilogue slot -> engine maps ('V' dve, 'A' act). Pool's tensor_scalar
# with accum_out is rejected by the TRN2 ISA, so Pool handles casts only.
# Group order must be V..., A... (finalize reduces contiguous ranges).
SUM_ENG = ["V", "V", "V", "V", "V", "V", "V", "V"]
CNT_ENG = ["V", "V", "A", "A", "A", "A", "A", "A"]
N_V_SUM = sum(1 for e in SUM_ENG if e == "V")
N_V_CNT = sum(1 for e in CNT_ENG if e == "V")
N_A_SUM = K - N_V_SUM
N_A_CNT = K - N_V_CNT
HW = 24  # per-half acc cols: 0-7 sums, 8-15 counts, 16-23 pm
ACC_W = 2 * HW


def build_kernel(do_compile=True, reps=1):
    nc = bacc_mod.Bacc()
    x_in = nc.declare_dram_parameter("x", [PPC * N, C], F32, isOutput=False)
    sn_out = nc.declare_dram_parameter("sn", [1, ACC_W * PPC], F32, isOutput=True)

    with tile.TileContext(nc) as tc, ExitStack() as ctx:
        consts = ctx.enter_context(tc.tile_pool(name="consts", bufs=1))
        xpool = ctx.enter_context(tc.tile_pool(name="xpool", bufs=2))
        xtpool = ctx.enter_context(tc.tile_pool(name="xtpool", bufs=4))
        dpool = ctx.enter_context(tc.tile_pool(name="dpool", bufs=4))
        small = ctx.enter_context(tc.tile_pool(name="small", bufs=8))
        trash = ctx.enter_context(tc.tile_pool(name="trash", bufs=12))
        accp = ctx.enter_context(tc.tile_pool(name="accp", bufs=4))
        psum = ctx.enter_context(tc.tile_pool(name="psum", bufs=6, space="PSUM"))
        psmall = ctx.enter_context(tc.tile_pool(name="psmall", bufs=1, space="PSUM"))

        # ---- one-time constants ----
        ct_one = consts.tile([NCLS, N], BF16, tag="ct1")
        nc.vector.memset(ct_one[:], 1.0)
        nc.gpsimd.affine_select(
            ct_one[:], ct_one[:], pattern=[[1, NCLS], [0, K]],
            compare_op=ALU.is_equal, fill=0.0, base=0, channel_multiplier=-1,
        )
        ct_a = consts.tile([NCLS, N], BF16, tag="cta")  # -L/2 * B
        nc.vector.memset(ct_a[:], -LBIG / 2)
        nc.gpsimd.affine_select(
            ct_a[:], ct_a[:], pattern=[[1, NCLS], [0, K]],
            compare_op=ALU.is_equal, fill=0.0, base=0, channel_multiplier=-1,
        )
        ident = consts.tile([128, 128], BF16, tag="ident")
        nc.vector.memset(ident[:], 1.0)
        nc.gpsimd.affine_select(
            ident[:], ident[:], pattern=[[1, 128]],
            compare_op=ALU.is_equal, fill=0.0, base=0, channel_multiplier=-1,
        )
        neghalf = consts.tile([1, 128], BF16, tag="neghalf")
        nc.vector.memset(neghalf[:], -0.5)
        ones_col = consts.tile([128, 1], F32, tag="ones_col")
        nc.vector.memset(ones_col[:], 1.0)

        # persistent cross-part psum strip
        fin_ps = psmall.tile([1, ACC_W * PPC], F32, tag="fin_ps")

        # ---- prefetch all input loads up front (dependency-free, so no
        # head-of-line blocking on the in-order SP queue); casts and xbar
        # transposes stay inline with each part's compute
        NPARTS = reps * PPC
        PRELOAD = min(6, NPARTS)
        parts = [pp for _ in range(reps) for pp in range(PPC)]
        xfs = {}

        def load(pi):
            p = parts[pi]
            xf = [xpool.tile([128, C], F32, tag="xf", name="xf", bufs=2 * (PRELOAD + 2))
                  for _ in range(HALVES)]
            for h in range(HALVES):
                nc.sync.dma_start(xf[h][:], x_in[p * N + 128 * h: p * N + 128 * (h + 1), :])
            xfs[pi] = xf

        for pi in range(PRELOAD):
            load(pi)

        pending_colsum = None
        for pi, p in enumerate(parts):
            xf = xfs.pop(pi)
            xb = [xpool.tile([128, C], BF16, tag="xb", name="xb", bufs=6)
                  for _ in range(HALVES)]
            for h in range(HALVES):
                nc.gpsimd.tensor_copy(xb[h][:], xf[h][:])
            xtb_all = xtpool.tile([128, 2 * N], BF16, tag="xtb", name="xtb", bufs=4)
            xtb_v = xtb_all[:].rearrange("q (a b) -> q a b", a=2, b=N)
            for h in range(HALVES):
                nc.sync.dma_start_transpose(
                    xtb_v[:, :, 128 * h: 128 * (h + 1)], xb[h][:]
                )
            xtb = [xtb_all[:, 0:N], xtb_all[:, N: 2 * N]]
            if pi + PRELOAD < NPARTS:
                load(pi + PRELOAD)

            # ---- squared norms: sqcol (f32, DVE mul-reduce) + sqrow bf16 ----
            sqcol = []
            for h in range(HALVES):
                sc = small.tile([128, 1], F32, tag="sqcol")
                st = trash.tile([128, C], BF16, tag="tr_sq")
                nc.scalar.activation(st[:], xb[h][:], ACTF.Square, accum_out=sc[:])
                sqcol.append(sc)
            sqcol_b = []
            for h in range(HALVES):
                scb = small.tile([128, 1], BF16, tag="sqcolb")
                nc.vector.tensor_copy(scb[:], sqcol[h][:])
                sqcol_b.append(scb)
            sqrow_ps = psmall.tile([1, N], BF16, tag="sqrow_ps")
            for h in range(HALVES):
                nc.tensor.transpose(
                    sqrow_ps[0:1, 128 * h: 128 * (h + 1)], sqcol_b[h][:], ident[:]
                )
            sqrow = small.tile([1, N], BF16, tag="sqrow")
            nc.vector.tensor_copy(sqrow[:], sqrow_ps[:])

            # ---- per half: gram + pollution; sqrt -> D' bf16; flip; spos ----
            acc = accp.tile([128, ACC_W], F32, tag="acc", name="acc")
            dmat = []
            argc = small.tile([128, 2 * K], F32, tag="argc")
            for h in range(HALVES):
                ps = psum.tile([128, N], F32, tag="ps")
                nc.tensor.matmul(
                    ps[:], xtb[0][:, 128 * h: 128 * (h + 1)], xtb[0][:],
                    start=True, stop=False,
                )
                nc.tensor.matmul(
                    ps[:], xtb[1][:, 128 * h: 128 * (h + 1)], xtb[1][:],
                    start=False, stop=False,
                )
                nc.tensor.matmul(
                    ps[:], neghalf[:, 0:128], sqrow[:], start=False, stop=False,
                )
                nc.tensor.matmul(
                    ps[:], ct_a[:, 128 * h: 128 * (h + 1)], ct_one[:],
                    start=False, stop=True,
                )
                dm = dpool.tile([128, N], BF16, tag="dmat")
                nc.scalar.activation(
                    dm[:], ps[:], ACTF.Sqrt, bias=sqcol[h][:], scale=-2.0,
                )
                dmat.append(dm)

                # own-class raw grams sit at -L/2 in psum1: min-reduce
                # picks them without any flip matmul (sqrt runs in parallel)
                spos = small.tile([128, K], F32, tag="spos")
                nc.vector.tensor_reduce(
                    spos[:],
                    ps[:].rearrange("q (g t) -> q t g", g=NCLS, t=K),
                    axis=mybir.AxisListType.X, op=ALU.min,
                )
                # argc[:, 8h:8h+8] = sqcol - 2*spos  (squared pos dists)
                nc.vector.tensor_scalar(
                    argc[:, K * h: K * (h + 1)], spos[:], -2.0, sqcol[h][:],
                    op0=ALU.mult, op1=ALU.add,
                )
            # previous part's acc column-sum goes on the PE queue here --
            # after this part's gram matmuls -- so those never stall behind
            # a colsum that is still waiting on the old epilogue's accums
            if pending_colsum is not None:
                prev_p, prev_acc = pending_colsum
                nc.tensor.matmul(
                    fin_ps[0:1, ACC_W * prev_p: ACC_W * (prev_p + 1)],
                    ones_col[:], prev_acc[:], start=True, stop=True,
                )

            # undo the -L/2 pollution (scale -2 => +L), clamp, sqrt, margin
            nc.vector.tensor_scalar(argc[:], argc[:], LBIG, EPS2, op0=ALU.subtract, op1=ALU.max)
            pm = small.tile([128, 2 * K], F32, tag="pm")
            nc.scalar.activation(pm[:], argc[:], ACTF.Sqrt)
            nc.vector.tensor_scalar(pm[:], pm[:], MARGIN, None, op0=ALU.add)
            # stash pm into acc cols 16-23 per half so the PE column-sum
            # yields per-(p,h,t) threshold sums for the finalize correction
            nc.vector.tensor_copy(
                acc[:].rearrange("q (h c) -> q h c", h=2, c=HW)[:, :, 16: 16 + K],
                pm[:].rearrange("q (h t) -> q h t", h=2, t=K),
            )

            # ---- epilogue: per (half, slot) one sum pass + one count pass ----
            for h in range(HALVES):
                pmh = pm[:, K * h: K * (h + 1)]
                for t in range(K):
                    a_sum = acc[:, HW * h + t: HW * h + t + 1]
                    a_cnt = acc[:, HW * h + 8 + t: HW * h + 9 + t]
                    if SUM_ENG[t] == "V":
                        o = trash.tile([128, N], BF16, tag="trS")
                        nc.vector.tensor_scalar(
                            o[:], dmat[h][:], pmh[:, t: t + 1], None,
                            op0=ALU.min, op1=ALU.add, accum_out=a_sum,
                        )
                    else:
                        o = trash.tile([128, N], BF16, tag="trS")
                        nc.scalar.activation(
                            o[:], dmat[h][:], ACTF.Relu,
                            bias=pmh[:, t: t + 1], scale=-1.0, accum_out=a_sum,
                        )
                    if CNT_ENG[t] == "V":
                        o = trash.tile([128, N], BF16, tag="trC")
                        nc.vector.tensor_scalar(
                            o[:], dmat[h][:], pmh[:, t: t + 1], None,
                            op0=ALU.is_lt, op1=ALU.add, accum_out=a_cnt,
                        )
                    else:
                        o = trash.tile([128, N], BF16, tag="trC")
                        nc.scalar.activation(
                            o[:], dmat[h][:], ACTF.Sign,
                            bias=pmh[:, t: t + 1], scale=-1.0, accum_out=a_cnt,
                        )

            # ---- cross-partition col sums via PE (deferred one part) ----
            pending_colsum = (p, acc)

        if pending_colsum is not None:
            prev_p, prev_acc = pending_colsum
            nc.tensor.matmul(
                fin_ps[0:1, ACC_W * prev_p: ACC_W * (prev_p + 1)],
                ones_col[:], prev_acc[:], start=True, stop=True,
            )

        # ---- dump raw per-part column sums; host does the final math ----
        fin = small.tile([1, ACC_W * PPC], F32, tag="fin")
        nc.vector.tensor_copy(fin[:], fin_ps[:])
        nc.sync.dma_start(sn_out[:], fin[:])

    if do_compile:
        nc.compile()
    return nc


_NC_CACHE = None


def _get_nc():
    global _NC_CACHE
    if _NC_CACHE is None:
        _NC_CACHE = build_kernel()
    return _NC_CACHE


def kernel(x: np.ndarray, y: np.ndarray) -> np.ndarray:
    from concourse.bass_utils import run_bass_kernel_spmd

    x = np.asarray(x)
    y = np.asarray(y)
    assert x.shape == (P_TOT, N, C) and y.shape == (P_TOT, N)
    expect = np.repeat(np.arange(NCLS, dtype=np.int64), K)
    assert np.array_equal(y, np.broadcast_to(expect, (P_TOT, N))), (
        "kernel requires y[p, i] == i // 8"
    )
    nc = _get_nc()
    xs = np.ascontiguousarray(x.reshape(NCORES, PPC * N, C).astype(np.float32))
    in_maps = [{"x": xs[i]} for i in range(NCORES)]
    res = run_bass_kernel_spmd(nc, in_maps, list(range(NCORES)))
    out = np.empty((P_TOT,), np.float32)
    for i in range(NCORES):
        fin = res.results[i]["sn"].reshape(PPC, HALVES, HW).astype(np.float64)
        sums = fin[:, :, 0:8]
        cnts = fin[:, :, 8:16]
        pms = fin[:, :, 16:24]
        S = (N * pms[:, :, :N_V_SUM].sum(2) - sums[:, :, :N_V_SUM].sum(2))
        if N_A_SUM:
            S += sums[:, :, N_V_SUM:].sum(2)
        Nc = cnts[:, :, :N_V_CNT].sum(2)
        if N_A_CNT:
            Nc += (cnts[:, :, N_V_CNT:].sum(2) + 128 * N * N_A_CNT) / 2
        S_p = S.sum(1)
        N_p = Nc.sum(1)
        out[i * PPC: (i + 1) * PPC] = np.where(
            N_p <= 0, 0.0, S_p / np.maximum(N_p, 1.0)
        ).astype(np.float32)
    return out


# revision 29
# speedup vs baseline: 1.0783x; 1.0335x over previous
"""BatchAllTripletLoss Trainium2 kernel.

Problem (hardcoded): x (64, 256, 256) f32, y (64, 256) int64 with
y[p, i] = i // 8 (32 classes x 8 members, uniform, identical across parts).
Output: per-part batch-all triplet loss, shape (64,) f32.

Math per part:
  D[i,j]  = euclidean distance matrix from x_p
  pos(i)  = 8 same-class columns (incl. self), neg(i) = 248 others
  loss_p  = mean over nonzero of relu(margin + D[i,j] - D[i,l]),
            j in pos(i), l in neg(i)

Device strategy (8 NeuronCores, 8 parts each, fully independent):
  - all x row loads prefetched up front on the SP DMA queue (they are
    dependency-free, so no head-of-line blocking); per part: cast to
    bf16 on Pool, transpose via DMA xbar (dma_start_transpose) -- the
    PE does no transposes.
  - gram via PE (bf16); -sq_row/2 rank-1 and -L/2 on same-class columns
    (rank-32) folded into one PSUM group; ACT sqrt (scale=-2,
    bias=sq_col) -> D' in BF16 (pos columns polluted to ~1024).
  - own-class raw grams sit at -L/2 in psum1, so a DVE min-reduce
    extracts them with NO second PSUM accumulation; true pos dists via
    one small clamped sqrt (clamp also guards the self slot, replacing
    the diagonal-eps matmul).
  - epilogue per (half, pos-slot): one sum pass + one count pass.
    DVE sum: accum Sum_l min(D', pm) (op1=add is the accum reducer);
    relu-sum recovered on the host as 256*pm_sum - Sum(min), with the
    per-anchor pm sums riding the same PE column-sum via pm columns in
    the acc tile.  Counts: DVE is_lt (2 slots) + ACT Sign (6 slots),
    balancing measured engine rates (DVE ~390ns/pass at the 1x
    CACHE_REDUCE rate -- accumulating tensor_scalar never enters the
    2x/4x DVE modes -- vs ACT ~590ns incl. accumulator read).
  - per-core output: raw [1, 384] column sums; host finishes the
    (S_p, N_p) reconstruction and division.
"""

import numpy as np
from contextlib import ExitStack

import concourse.bass as bass
import concourse.bacc as bacc_mod
import concourse.mybir as mybir
import concourse.tile as tile

F32 = mybir.dt.float32
BF16 = mybir.dt.bfloat16
ALU = mybir.AluOpType
ACTF = mybir.ActivationFunctionType

# problem constants
P_TOT, N, C = 64, 256, 256
K, NCLS = 8, 32
MARGIN = 0.2
NCORES = 8
PPC = P_TOT // NCORES  # parts per core
HALVES = 2
LBIG = float(2 << 19)  # 2^20 pollution offset
EPS2 = 0.04  # clamp floor for pos squared-dists (self slot)

# epilogue slot -> engine maps ('V' dve, 'A' act). Pool's tensor_scalar
# with accum_out is rejected by the TRN2 ISA, so Pool handles casts only.
# Group order must be V..., A... (finalize reduces contiguous ranges).
SUM_ENG = ["V", "V", "V", "V", "V", "V", "V", "V"]
CNT_ENG = ["V", "V", "A", "A", "A", "A", "A", "A"]
N_V_SUM = sum(1 for e in SUM_ENG if e == "V")
N_V_CNT = sum(1 for e in CNT_ENG if e == "V")
N_A_SUM = K - N_V_SUM
N_A_CNT = K - N_V_CNT
HW = 24  # per-half acc cols: 0-7 sums, 8-15 counts, 16-23 pm
ACC_W = 2 * HW


def build_kernel(do_compile=True, reps=1):
    nc = bacc_mod.Bacc()
    x_in = nc.declare_dram_parameter("x", [PPC * N, C], F32, isOutput=False)
    sn_out = nc.declare_dram_parameter("sn", [1, ACC_W * PPC], F32, isOutput=True)

    with tile.TileContext(nc) as tc, ExitStack() as ctx:
        consts = ctx.enter_context(tc.tile_pool(name="consts", bufs=1))
        xpool = ctx.enter_context(tc.tile_pool(name="xpool", bufs=2))
        xtpool = ctx.enter_context(tc.tile_pool(name="xtpool", bufs=4))
        dpool = ctx.enter_context(tc.tile_pool(name="dpool", bufs=4))
        small = ctx.enter_context(tc.tile_pool(name="small", bufs=8))
        trash = ctx.enter_context(tc.tile_pool(name="trash", bufs=16))
        accp = ctx.enter_context(tc.tile_pool(name="accp", bufs=4))
        psum = ctx.enter_context(tc.tile_pool(name="psum", bufs=6, space="PSUM"))
        psmall = ctx.enter_context(tc.tile_pool(name="psmall", bufs=1, space="PSUM"))

        # ---- one-time constants ----
        ct_one = consts.tile([NCLS, N], BF16, tag="ct1")
        nc.vector.memset(ct_one[:], 1.0)
        nc.gpsimd.affine_select(
            ct_one[:], ct_one[:], pattern=[[1, NCLS], [0, K]],
            compare_op=ALU.is_equal, fill=0.0, base=0, channel_multiplier=-1,
        )
        ct_a = consts.tile([NCLS, N], BF16, tag="cta")  # -L/2 * B
        nc.vector.memset(ct_a[:], -LBIG / 2)
        nc.gpsimd.affine_select(
            ct_a[:], ct_a[:], pattern=[[1, NCLS], [0, K]],
            compare_op=ALU.is_equal, fill=0.0, base=0, channel_multiplier=-1,
        )
        ident = consts.tile([128, 128], BF16, tag="ident")
        nc.vector.memset(ident[:], 1.0)
        nc.gpsimd.affine_select(
            ident[:], ident[:], pattern=[[1, 128]],
            compare_op=ALU.is_equal, fill=0.0, base=0, channel_multiplier=-1,
        )
        neghalf = consts.tile([1, 128], BF16, tag="neghalf")
        nc.vector.memset(neghalf[:], -0.5)
        ones_col = consts.tile([128, 1], F32, tag="ones_col")
        nc.vector.memset(ones_col[:], 1.0)

        # persistent cross-part psum strip
        fin_ps = psmall.tile([1, ACC_W * PPC], F32, tag="fin_ps")

        # ---- prefetch all input loads up front (dependency-free, so no
        # head-of-line blocking on the in-order SP queue); casts and xbar
        # transposes stay inline with each part's compute
        NPARTS = reps * PPC
        PRELOAD = min(6, NPARTS)
        parts = [pp for _ in range(reps) for pp in range(PPC)]
        xfs = {}

        def load(pi):
            p = parts[pi]
            xf = [xpool.tile([128, C], F32, tag="xf", name="xf", bufs=2 * (PRELOAD + 2))
                  for _ in range(HALVES)]
            for h in range(HALVES):
                nc.sync.dma_start(xf[h][:], x_in[p * N + 128 * h: p * N + 128 * (h + 1), :])
            xfs[pi] = xf

        for pi in range(PRELOAD):
            load(pi)

        pending_colsum = None
        for pi, p in enumerate(parts):
            xf = xfs.pop(pi)
            xb = [xpool.tile([128, C], BF16, tag="xb", name="xb", bufs=6)
                  for _ in range(HALVES)]
            for h in range(HALVES):
                nc.gpsimd.tensor_copy(xb[h][:], xf[h][:])
            xtb_all = xtpool.tile([128, 2 * N], BF16, tag="xtb", name="xtb", bufs=4)
            xtb_v = xtb_all[:].rearrange("q (a b) -> q a b", a=2, b=N)
            for h in range(HALVES):
                nc.sync.dma_start_transpose(
                    xtb_v[:, :, 128 * h: 128 * (h + 1)], xb[h][:]
                )
            xtb = [xtb_all[:, 0:N], xtb_all[:, N: 2 * N]]
            if pi + PRELOAD < NPARTS:
                load(pi + PRELOAD)

            # ---- squared norms: sqcol (f32, DVE mul-reduce) + sqrow bf16 ----
            sqcol = []
            for h in range(HALVES):
                sc = small.tile([128, 1], F32, tag="sqcol")
                st = trash.tile([128, C], BF16, tag="tr_sq")
                nc.scalar.activation(st[:], xb[h][:], ACTF.Square, accum_out=sc[:])
                sqcol.append(sc)
            sqcol_b = []
            for h in range(HALVES):
                scb = small.tile([128, 1], BF16, tag="sqcolb")
                nc.vector.tensor_copy(scb[:], sqcol[h][:])
                sqcol_b.append(scb)
            sqrow_ps = psmall.tile([1, N], BF16, tag="sqrow_ps")
            for h in range(HALVES):
                nc.tensor.transpose(
                    sqrow_ps[0:1, 128 * h: 128 * (h + 1)], sqcol_b[h][:], ident[:]
                )
            sqrow = small.tile([1, N], BF16, tag="sqrow")
            nc.vector.tensor_copy(sqrow[:], sqrow_ps[:])

            # ---- per half: gram + pollution; sqrt -> D' bf16; flip; spos ----
            acc = accp.tile([128, ACC_W], F32, tag="acc", name="acc")
            dmat = []
            argc = small.tile([128, 2 * K], F32, tag="argc")
            for h in range(HALVES):
                ps = psum.tile([128, N], F32, tag="ps")
                nc.tensor.matmul(
                    ps[:], xtb[0][:, 128 * h: 128 * (h + 1)], xtb[0][:],
                    start=True, stop=False,
                )
                nc.tensor.matmul(
                    ps[:], xtb[1][:, 128 * h: 128 * (h + 1)], xtb[1][:],
                    start=False, stop=False,
                )
                nc.tensor.matmul(
                    ps[:], neghalf[:, 0:128], sqrow[:], start=False, stop=False,
                )
                nc.tensor.matmul(
                    ps[:], ct_a[:, 128 * h: 128 * (h + 1)], ct_one[:],
                    start=False, stop=True,
                )
                dm = dpool.tile([128, N], BF16, tag="dmat")
                nc.scalar.activation(
                    dm[:], ps[:], ACTF.Sqrt, bias=sqcol[h][:], scale=-2.0,
                )
                dmat.append(dm)

                # own-class raw grams sit at -L/2 in psum1: min-reduce
                # picks them without any flip matmul (sqrt runs in parallel)
                spos = small.tile([128, K], F32, tag="spos")
                nc.vector.tensor_reduce(
                    spos[:],
                    ps[:].rearrange("q (g t) -> q t g", g=NCLS, t=K),
                    axis=mybir.AxisListType.X, op=ALU.min,
                )
                # argc[:, 8h:8h+8] = sqcol - 2*spos  (squared pos dists)
                nc.vector.tensor_scalar(
                    argc[:, K * h: K * (h + 1)], spos[:], -2.0, sqcol[h][:],
                    op0=ALU.mult, op1=ALU.add,
                )
            # previous part's acc column-sum goes on the PE queue here --
            # after this part's gram matmuls -- so those never stall behind
            # a colsum that is still waiting on the old epilogue's accums
            if pending_colsum is not None:
                prev_p, prev_acc = pending_colsum
                nc.tensor.matmul(
                    fin_ps[0:1, ACC_W * prev_p: ACC_W * (prev_p + 1)],
                    ones_col[:], prev_acc[:], start=True, stop=True,
                )

            # undo the -L/2 pollution (scale -2 => +L), clamp, sqrt, margin
            nc.vector.tensor_scalar(argc[:], argc[:], LBIG, EPS2, op0=ALU.subtract, op1=ALU.max)
            pm = small.tile([128, 2 * K], F32, tag="pm")
            nc.scalar.activation(pm[:], argc[:], ACTF.Sqrt)
            nc.vector.tensor_scalar(pm[:], pm[:], MARGIN, None, op0=ALU.add)
            # stash pm into acc cols 16-23 per half so the PE column-sum
            # yields per-(p,h,t) threshold sums for the finalize correction
            nc.vector.tensor_copy(
                acc[:].rearrange("q (h c) -> q h c", h=2, c=HW)[:, :, 16: 16 + K],
                pm[:].rearrange("q (h t) -> q h t", h=2, t=K),
            )

            # ---- epilogue: per (half, slot) one sum pass + one count pass ----
            for h in range(HALVES):
                pmh = pm[:, K * h: K * (h + 1)]
                for t in range(K):
                    a_sum = acc[:, HW * h + t: HW * h + t + 1]
                    a_cnt = acc[:, HW * h + 8 + t: HW * h + 9 + t]
                    if SUM_ENG[t] == "V":
                        o = trash.tile([128, N], BF16, tag="trS")
                        nc.vector.tensor_scalar(
                            o[:], dmat[h][:], pmh[:, t: t + 1], None,
                            op0=ALU.min, op1=ALU.add, accum_out=a_sum,
                        )
                    else:
                        o = trash.tile([128, N], BF16, tag="trS")
                        nc.scalar.activation(
                            o[:], dmat[h][:], ACTF.Relu,
                            bias=pmh[:, t: t + 1], scale=-1.0, accum_out=a_sum,
                        )
                    if CNT_ENG[t] == "V":
                        o = trash.tile([128, N], BF16, tag="trCV", name="trCV")
                        nc.vector.tensor_scalar(
                            o[:], dmat[h][:], pmh[:, t: t + 1], None,
                            op0=ALU.is_lt, op1=ALU.add, accum_out=a_cnt,
                        )
                    else:
                        o = trash.tile([128, N], BF16, tag="trCA", name="trCA")
                        nc.scalar.activation(
                            o[:], dmat[h][:], ACTF.Sign,
                            bias=pmh[:, t: t + 1], scale=-1.0, accum_out=a_cnt,
                        )

            # ---- cross-partition col sums via PE (deferred one part) ----
            pending_colsum = (p, acc)

        if pending_colsum is not None:
            prev_p, prev_acc = pending_colsum
            nc.tensor.matmul(
                fin_ps[0:1, ACC_W * prev_p: ACC_W * (prev_p + 1)],
                ones_col[:], prev_acc[:], start=True, stop=True,
            )

        # ---- dump raw per-part column sums; host does the final math ----
        fin = small.tile([1, ACC_W * PPC], F32, tag="fin")
        nc.vector.tensor_copy(fin[:], fin_ps[:])
        nc.sync.dma_start(sn_out[:], fin[:])

    if do_compile:
        nc.compile()
    return nc


_NC_CACHE = None


def _get_nc():
    global _NC_CACHE
    if _NC_CACHE is None:
        _NC_CACHE = build_kernel()
    return _NC_CACHE


def kernel(x: np.ndarray, y: np.ndarray) -> np.ndarray:
    from concourse.bass_utils import run_bass_kernel_spmd

    x = np.asarray(x)
    y = np.asarray(y)
    assert x.shape == (P_TOT, N, C) and y.shape == (P_TOT, N)
    expect = np.repeat(np.arange(NCLS, dtype=np.int64), K)
    assert np.array_equal(y, np.broadcast_to(expect, (P_TOT, N))), (
        "kernel requires y[p, i] == i // 8"
    )
    nc = _get_nc()
    xs = np.ascontiguousarray(x.reshape(NCORES, PPC * N, C).astype(np.float32))
    in_maps = [{"x": xs[i]} for i in range(NCORES)]
    res = run_bass_kernel_spmd(nc, in_maps, list(range(NCORES)))
    out = np.empty((P_TOT,), np.float32)
    for i in range(NCORES):
        fin = res.results[i]["sn"].reshape(PPC, HALVES, HW).astype(np.float64)
        sums = fin[:, :, 0:8]
        cnts = fin[:, :, 8:16]
        pms = fin[:, :, 16:24]
        S = (N * pms[:, :, :N_V_SUM].sum(2) - sums[:, :, :N_V_SUM].sum(2))
        if N_A_SUM:
            S += sums[:, :, N_V_SUM:].sum(2)
        Nc = cnts[:, :, :N_V_CNT].sum(2)
        if N_A_CNT:
            Nc += (cnts[:, :, N_V_CNT:].sum(2) + 128 * N * N_A_CNT) / 2
        S_p = S.sum(1)
        N_p = Nc.sum(1)
        out[i * PPC: (i + 1) * PPC] = np.where(
            N_p <= 0, 0.0, S_p / np.maximum(N_p, 1.0)
        ).astype(np.float32)
    return out
